# revision 1
# baseline (speedup 1.0000x reference)
"""Trainium2 Bass kernel for nn_MultiHeaded_4080218931880.

Multi-headed attention with the reference's *raw reshape* head split:
    q = from @ Wq + bq                      # (B, F, HD)
    q_r = q.reshape(B, H, D, F)             # raw row-major reshape
    score = einsum('bhdf,bhdt->bhft', q_r, k_r) * alpha
    probs = softmax(score + (1-mask)*NEG, axis=-1)
    out = einsum('bhft,bhdt->bhdf', probs, v_r).reshape(B, H*D, F)

Because the reshape is raw, head h only touches rows [2*D*h, 2*D*(h+1))
of the (F, HD) projection output, and the per-head (D, F) matrix is just
that row block flattened row-major.  So the 32 (b, h) pairs are fully
independent: shard 4 pairs per NeuronCore over 8 cores (pairs of one
batch stay on one core).

Per-core device program, two phases (all matmuls bf16, fp32 PSUM):

Phase P (projections, all 4 pairs):
  q/k/v = x @ W + b with x.T (pre-transposed on host) as stationary and
  W as moving; bias folded in as a K=1 ones-matmul that opens the PSUM
  accumulation group; alpha folded into k's PSUM eviction.  Each block
  is bounced through DRAM to realize the raw (2D, HD) -> (D, 2*HD)
  reshape (rows 2d', 2d'+1 are adjacent in DRAM so the read-back is
  contiguous).  v_r carries an extra ones row (row D) so the PE
  transposes produce v_r.T chunks WITH the ones column that later
  accumulates the softmax denominator.

Phase A (attention, per pair):
  score computed TRANSPOSED in 16 chunks (128 t' x 2048 f'):
  lhsT = k_r chunk (64, 128), rhs = q_r (64, 2048).  exp on ScalarE
  straight out of PSUM into bf16 SBUF tiles.  Context: lhsT = v_r.T
  chunk (128, 65), rhs = exp chunks, accumulated over the 16 chunks in
  PSUM; row 64 of the accumulator is the softmax denominator.
  Normalize via reciprocal + K=1 fp32 broadcast matmul + VectorE mul.
"""

import numpy as np
from contextlib import ExitStack

import concourse.bass as bass
import concourse.bacc as bacc
import concourse.tile as tile
from concourse import mybir
from concourse.bass_utils import run_bass_kernel_spmd
from concourse.masks import make_identity

BF16 = mybir.dt.bfloat16
F32 = mybir.dt.float32
NP_BF16 = mybir.dt.np(mybir.dt.bfloat16)

# Problem dims (hardcoded; harness runs kernel.py standalone).
B, F, T, C = 2, 2048, 2048, 1024
H, D = 16, 64
HD = H * D
ALPHA = 1.0 / np.sqrt(np.float32(D)).astype(np.float32)
NEG = -100000.0
N_CORES = 8
NPAIR = (B * H) // N_CORES  # 4 (b,h) pairs per core
P = 128

REAL_DIMS = dict(npair=NPAIR, c=C, hd=HD, d=D, f=F, t=T)


def _nsl(total, step):
    """Split [0, total) into <=step slices (matmul moving free-dim limit)."""
    return [(s, min(s + step, total)) for s in range(0, total, step)]


def build_program(has_mask=False, has_bias=True, dims=None, exp_bufs=None):
    dm = dims or REAL_DIMS
    npair, c, hd, d, f, t = (
        dm["npair"], dm["c"], dm["hd"], dm["d"], dm["f"], dm["t"],
    )
    bh = 2 * d          # row-block height of x per (b,h) pair
    ncc = c // P        # contraction chunks for projections
    nch = t // P        # t' chunks for attention
    NB = 512            # matmul PSUM-write limit: one 2KB bank (512 f32)
    NF = 512

    nc = bacc.Bacc(None, target_bir_lowering=False, debug=True)
    # x and W arrive pre-permuted to their exact SBUF layouts (partition
    # dim outermost), so every load DMA is fully contiguous
    xfT = nc.declare_dram_parameter("xfT", [npair, P, ncc, bh], BF16, isOutput=False)
    xtT = nc.declare_dram_parameter("xtT", [npair, P, ncc, bh], BF16, isOutput=False)
    wq = nc.declare_dram_parameter("wq", [P, ncc, hd], BF16, isOutput=False)
    wk = nc.declare_dram_parameter("wk", [P, ncc, hd], BF16, isOutput=False)
    wv = nc.declare_dram_parameter("wv", [P, ncc, hd], BF16, isOutput=False)
    bq = nc.declare_dram_parameter("bq", [1, hd], BF16, isOutput=False)
    bk = nc.declare_dram_parameter("bk", [1, hd], BF16, isOutput=False)
    bv = nc.declare_dram_parameter("bv", [1, hd], BF16, isOutput=False)
    mbT = None
    if has_mask:
        # (1 - mask[b]).T * NEG / ALPHA is NOT needed: alpha lives in k, so
        # the additive bias is exactly (1 - mask[b]).T * NEG.
        mbT = nc.declare_dram_parameter("mbT", [t, f], BF16, isOutput=False)
    out_d = nc.declare_dram_parameter("out", [npair, d, f], F32, isOutput=True)

    with tile.TileContext(nc) as tc, ExitStack() as ctx:
        # ---- pools resident for the whole kernel ----
        const = ctx.enter_context(tc.tile_pool(name="const", bufs=1))
        wpool = ctx.enter_context(tc.tile_pool(name="wpool", bufs=1))
        rqk = ctx.enter_context(tc.tile_pool(name="rqk", bufs=2 * npair))
        vpool = ctx.enter_context(tc.tile_pool(name="vpool", bufs=npair * nch))
        dpool = ctx.enter_context(tc.tile_pool(name="dpool", bufs=3, space="DRAM"))

        if has_bias:
            ones_row = const.tile([1, P], BF16)
            nc.vector.memset(ones_row[:], 1.0)
        # ones row at base partition d: pairs with the reciprocal row (also
        # at partition d) in the K=1 broadcast matmul (matmul requires equal
        # base partitions for lhsT and rhs); bf16 so the broadcast streams
        # at 1 cycle/row instead of fp32's 4
        ones_at_d = const.tile([d + 1, d], BF16)
        nc.vector.memset(ones_at_d[d:d + 1, :], 1.0)
        ident = const.tile([d + 1, d + 1], BF16)
        make_identity(nc, ident[:])

        w_s, b_s = {}, {}

        def load_weights():
            for name, wd, bd in (("q", wq, bq), ("k", wk, bk), ("v", wv, bv)):
                wt = wpool.tile([P, ncc, hd], BF16, tag=f"w{name}")
                # per-chunk loads so the first projection matmul only waits
                # for one 256KB transfer, not the whole 2MB weight
                for kc in range(ncc):
                    nc.sync.dma_start(out=wt[:, kc, :], in_=wd[:, kc, :])
                w_s[name] = wt
                if has_bias:
                    bt = wpool.tile([1, hd], BF16, tag=f"b{name}")
                    nc.sync.dma_start(out=bt[:], in_=bd[:])
                    b_s[name] = bt

        r_all = [{} for _ in range(npair)]
        vones_all = [[] for _ in range(npair)]
        cx_hold = {}
        fh = f // 2

        xpool = ctx.enter_context(tc.tile_pool(name="xpool", bufs=2))
        blkpool = ctx.enter_context(tc.tile_pool(name="blkpool", bufs=3))
        rv = ctx.enter_context(tc.tile_pool(name="rv", bufs=2))
        if exp_bufs is None:
            exp_bufs = 10 if has_mask else 12
        epool = ctx.enter_context(tc.tile_pool(name="epool", bufs=exp_bufs))
        opool = ctx.enter_context(tc.tile_pool(name="opool", bufs=2))
        spool = ctx.enter_context(tc.tile_pool(name="spool", bufs=1))
        mpool = None
        if has_mask:
            mpool = ctx.enter_context(tc.tile_pool(name="mpool", bufs=4))
        # PSUM: mix slots (128, hd) f32 [2 banks x 2] serve projections,
        # score halves and the transpose blocks; ctx accumulator [4 banks].
        pp_mix = ctx.enter_context(tc.tile_pool(name="pp_mix", bufs=2, space="PSUM"))
        pp_ctx = ctx.enter_context(tc.tile_pool(name="pp_ctx", bufs=1, space="PSUM"))

        def emit_proj(j):
            """Projections + reshape + v transposes for pair j (generator:
            yields between chunks so the driver can interleave it with the
            previous pair's attention in PE program order)."""
            xf_s = xpool.tile([P, ncc, bh], BF16, tag="xf")
            nc.sync.dma_start(out=xf_s[:], in_=xfT[j])
            xt_s = xpool.tile([P, ncc, bh], BF16, tag="xt")
            nc.sync.dma_start(out=xt_s[:], in_=xtT[j])
            yield
            for name, x_s in (("q", xf_s), ("k", xt_s), ("v", xt_s)):
                pj = pp_mix.tile([bh, hd], F32, tag="mix")
                if has_bias:
                    for ns, ne in _nsl(hd, NB):
                        nc.tensor.matmul(
                            pj[:, ns:ne], ones_row[:, :bh],
                            b_s[name][:, ns:ne],
                            start=True, stop=False,
                        )
                for kc in range(ncc):
                    first = kc == 0 and not has_bias
                    last = kc == ncc - 1
                    for ns, ne in _nsl(hd, NB):
                        nc.tensor.matmul(
                            pj[:, ns:ne], x_s[:, kc, :],
                            w_s[name][:, kc, ns:ne],
                            start=first, stop=last,
                        )
                    if kc % 3 == 2:
                        yield
                blk = blkpool.tile([bh, hd], BF16, tag="blk")
                if name == "k":
                    # fold alpha into k so exp needs no input scale
                    nc.vector.tensor_scalar_mul(blk[:], pj[:], float(ALPHA))
                else:
                    nc.vector.tensor_copy(blk[:], pj[:])
                # DRAM bounce realizes the raw (2d, hd)->(d, 2*hd) reshape:
                # rows 2d', 2d'+1 are adjacent in DRAM, so the read-back is
                # contiguous per partition.
                dsc = dpool.tile([bh, hd], BF16, tag="dsc")
                nc.sync.dma_start(out=dsc[:], in_=blk[:])
                if name == "v":
                    r = rv.tile([d + 1, 2 * hd], BF16, tag="rv")
                else:
                    r = rqk.tile([d, 2 * hd], BF16, tag=f"r{name}")
                nc.sync.dma_start(
                    out=r[0:d, :],
                    in_=dsc[:].rearrange("(d two) n -> d (two n)", two=2),
                )
                r_all[j][name] = r
                yield
            # ones row -> transposes carry the denominator column
            r_v = r_all[j]["v"]
            nc.vector.memset(r_v[d:d + 1, :], 1.0)
            # transposes in 4-chunk blocks so each PSUM mix-slot hold is
            # short (a long hold single-buffers the score pipeline);
            # inner dim padded to d+2 so bf16 PSUM slices stay 4B-aligned
            grp = 4
            for tg in range(0, nch, grp):
                gn = min(grp, nch - tg)
                vt_ps = pp_mix.tile([P, grp, d + 2], BF16, tag="mix")
                for ti in range(gn):
                    tcb = tg + ti
                    nc.tensor.transpose(
                        vt_ps[:, ti, 0:d + 1],
                        r_v[:, tcb * P:(tcb + 1) * P],
                        ident[:],
                    )
                    vo = vpool.tile([P, d + 1], BF16, tag="vones")
                    nc.vector.tensor_copy(vo[:], vt_ps[:, ti, 0:d + 1])
                    vones_all[j].append(vo)
                yield

        def emit_attn(j):
            """Attention for pair j: per t'-chunk, two score halves + exp,
            then the two ctx matmuls for that chunk (generator: yields per
            chunk)."""
            r_q, r_k = r_all[j]["q"], r_all[j]["k"]
            ps_cx = pp_ctx.tile([d + 1, f], F32, tag="cx")
            for tcb in range(nch):
                exs = []
                for hf in range(2):
                    ps_sc = pp_mix.tile([P, fh], F32, tag="mix")
                    for ns, ne in _nsl(fh, NB):
                        nc.tensor.matmul(
                            ps_sc[:, ns:ne],
                            r_k[:, tcb * P:(tcb + 1) * P],
                            r_q[:, hf * fh + ns:hf * fh + ne],
                            start=True, stop=True,
                        )
                    if has_mask:
                        mt = mpool.tile([P, fh], BF16, tag="mb")
                        nc.sync.dma_start(
                            out=mt[:],
                            in_=mbT[tcb * P:(tcb + 1) * P, hf * fh:(hf + 1) * fh],
                        )
                        nc.vector.tensor_add(ps_sc[:], ps_sc[:], mt[:])
                    ex = epool.tile([P, fh], BF16, tag="exp")
                    nc.scalar.activation(
                        ex[:], ps_sc[:], mybir.ActivationFunctionType.Exp
                    )
                    exs.append(ex)
                # PSUM accumulation groups work on 2KB zero regions (512
                # f32): start/stop must be set on the first/last write of
                # each region, not per slice.
                REG = 512
                for hf in range(2):
                    for ns, ne in _nsl(fh, NB):
                        gs, ge = hf * fh + ns, hf * fh + ne
                        nc.tensor.matmul(
                            ps_cx[:, gs:ge],
                            vones_all[j][tcb][:],
                            exs[hf][:, ns:ne],
                            start=(tcb == 0 and gs % REG == 0),
                            stop=(tcb == nch - 1 and (ge % REG == 0 or ge == f)),
                        )
                yield
            # ctx_t rows 0..d-1 hold the normalized output; row d is
            # scratch for the reciprocal of the softmax denominator.
            # eagerly evacuate the accumulator to SBUF so the 4-bank ctx
            # PSUM slot frees for the next pair; normalization happens
            # lazily, interleaved with the next pair's attention.
            cx_sb = opool.tile([d + 1, f], F32, tag="ctx")
            nc.vector.tensor_copy(cx_sb[:], ps_cx[:])
            cx_hold[j] = cx_sb
            yield

        def emit_norm(j):
            """Normalize pair j's evacuated accumulator and store it."""
            cx_sb = cx_hold[j]
            nc.vector.reciprocal(cx_sb[d:d + 1, :], cx_sb[d:d + 1, :])
            # bf16 copy of 1/S (same partition), then K=1 bf16 broadcast
            # matmuls; the ~0.4% bf16 error on 1/S is well inside budget
            rc_bf = spool.tile([d + 1, f], BF16, tag="rcb")
            nc.vector.tensor_copy(rc_bf[d:d + 1, :], cx_sb[d:d + 1, :])
            yield
            bc_sb = spool.tile([d, f], F32, tag="bc")
            for hs, he in _nsl(f, min(fh, 1024)):
                ps_bc = pp_mix.tile([d, min(fh, 1024)], F32, tag="mix")
                for ns, ne in _nsl(he - hs, NB):
                    nc.tensor.matmul(
                        ps_bc[:, ns:ne], ones_at_d[d:d + 1, :],
                        rc_bf[d:d + 1, hs + ns:hs + ne],
                        start=True, stop=True,
                    )
                nc.vector.tensor_copy(bc_sb[:, hs:he], ps_bc[:, 0:he - hs])
                yield
            nc.vector.tensor_mul(cx_sb[0:d, :], cx_sb[0:d, :], bc_sb[:])
            nc.sync.dma_start(out=out_d[j], in_=cx_sb[0:d, :])
            yield

        # software pipeline: pair j's attention interleaved (in program
        # order, hence in each engine's instruction stream) with pair j+1's
        # projections and pair j-1's normalization.
        pg0 = emit_proj(0)
        next(pg0)        # pair-0 x loads issue before the weight DMAs
        load_weights()
        for _ in pg0:
            pass
        ng = None
        for j in range(npair):
            pg = emit_proj(j + 1) if j + 1 < npair else None
            for _ in emit_attn(j):
                if pg is not None:
                    next(pg, None)
                if ng is not None:
                    next(ng, None)
            if pg is not None:
                for _ in pg:
                    pass
            if ng is not None:
                for _ in ng:
                    pass
            ng = emit_norm(j)
        for _ in ng:
            pass

    nc.finalize()
    return nc


_PROGRAM_CACHE = {}
TRACE = False
LAST_RESULTS = None


def _get_program(has_mask, has_bias):
    key = (has_mask, has_bias)
    if key not in _PROGRAM_CACHE:
        _PROGRAM_CACHE[key] = build_program(has_mask=has_mask, has_bias=has_bias)
    return _PROGRAM_CACHE[key]


def kernel(**inputs):
    from_tensor = np.asarray(inputs["from_tensor"], np.float32)
    to_tensor = np.asarray(inputs["to_tensor"], np.float32)
    mask = np.asarray(inputs["mask"], np.float32)

    def wprep(w):
        # (C, HD) -> (P, C//P, HD): the device SBUF layout, so the weight
        # chunk DMAs are fully contiguous
        w = np.asarray(w, np.float32).astype(NP_BF16)
        return np.ascontiguousarray(
            w.reshape(C // 128, 128, HD).transpose(1, 0, 2)
        )

    wq = wprep(inputs["Wq"])
    wk = wprep(inputs["Wk"])
    wv = wprep(inputs["Wv"])
    bqv = np.asarray(inputs["bq"], np.float32).astype(NP_BF16).reshape(1, HD)
    bkv = np.asarray(inputs["bk"], np.float32).astype(NP_BF16).reshape(1, HD)
    bvv = np.asarray(inputs["bv"], np.float32).astype(NP_BF16).reshape(1, HD)

    mb = (1.0 - mask) * NEG  # (B, F, T) additive mask bias
    has_mask = bool(np.any(mb != 0.0))
    has_bias = bool(
        np.any(inputs["bq"]) or np.any(inputs["bk"]) or np.any(inputs["bv"])
    )
    nc = _get_program(has_mask, has_bias)

    bh = 2 * D

    def xprep(x, p):
        # block (bh, C) -> transpose -> (P, C//P, bh) SBUF layout
        xb = x[p // H, (p % H) * bh:(p % H + 1) * bh, :].T.astype(NP_BF16)
        return np.ascontiguousarray(
            xb.reshape(C // 128, 128, bh).transpose(1, 0, 2)
        )

    in_maps = []
    for core in range(N_CORES):
        pairs = [4 * core + jj for jj in range(NPAIR)]
        b = pairs[0] // H
        xf = np.stack([xprep(from_tensor, p) for p in pairs])
        xt = np.stack([xprep(to_tensor, p) for p in pairs])
        m = {
            "xfT": xf, "xtT": xt,
            "wq": wq, "wk": wk, "wv": wv,
            "bq": bqv, "bk": bkv, "bv": bvv,
        }
        if has_mask:
            m["mbT"] = np.ascontiguousarray(mb[b].T).astype(NP_BF16)
        in_maps.append(m)

    res = run_bass_kernel_spmd(
        nc, in_maps, core_ids=list(range(N_CORES)), trace=TRACE
    )
    global LAST_RESULTS
    LAST_RESULTS = res

    out = np.empty((B, HD, F), np.float32)
    for core in range(N_CORES):
        o = res.results[core]["out"]
        for jj in range(NPAIR):
            p = 4 * core + jj
            b, h = p // H, p % H
            out[b, h * D:(h + 1) * D, :] = o[jj]
    return out



# revision 2
# speedup vs baseline: 1.0507x; 1.0507x over previous
"""Trainium2 Bass kernel for nn_MultiHeaded_4080218931880.

Multi-headed attention with the reference's *raw reshape* head split:
    q = from @ Wq + bq                      # (B, F, HD)
    q_r = q.reshape(B, H, D, F)             # raw row-major reshape
    score = einsum('bhdf,bhdt->bhft', q_r, k_r) * alpha
    probs = softmax(score + (1-mask)*NEG, axis=-1)
    out = einsum('bhft,bhdt->bhdf', probs, v_r).reshape(B, H*D, F)

Because the reshape is raw, head h only touches rows [2*D*h, 2*D*(h+1))
of the (F, HD) projection output, so the 32 (b, h) pairs are fully
independent: 4 pairs per NeuronCore over 8 cores.

Fast path (mask all-ones, zero biases — the shipped problem instance):

  Projections via fp8 e4m3 DoubleRow, 3-term split (x8@W8 + xr@W8 +
  x8@Wr with W prescaled by WS=64 to stay in e4m3 normal range; the
  1/WS folds into the eviction scalar ops).  Terms are paired across
  adjacent c-chunks of the same kind, so each DoubleRow instruction
  contracts K_eff=256 at 0.5 cycles/row — 25% cheaper than bf16 and
  slightly MORE accurate (residual term cancels the x quantization).

  Score via fp8 DoubleRow 4-term split: q -> q8+qr, k -> k8+kr (DVE
  evictions straight from the projection PSUM), stacked [q8;qr] on
  partitions and [k8;k8],[kr;kr] on the two DoubleRow j-subtiles.  One
  instruction per 512-slice computes exact-to-~fp16^2 scores at HALF
  the bf16 PE cost.  alpha is applied for free via the exp
  activation's scale parameter.

  ctx stays bf16 (exp tiles bf16, v bf16 with a ones row so the ctx
  accumulator's row D carries the softmax denominator).  The per-pair
  (D+1, F) accumulator is DMA'd out raw; the final divide by the
  denominator row happens on host during the gather/unshard step.

Fallback path (general mask/bias) keeps the previous all-bf16 program.
"""

import numpy as np
from contextlib import ExitStack

import concourse.bass as bass
import concourse.bacc as bacc
import concourse.tile as tile
from concourse import mybir
from concourse.bass_utils import run_bass_kernel_spmd
from concourse.masks import make_identity

BF16 = mybir.dt.bfloat16
F32 = mybir.dt.float32
FP8 = mybir.dt.float8e4
NP_BF16 = mybir.dt.np(mybir.dt.bfloat16)
NP_FP8 = mybir.dt.np(mybir.dt.float8e4)

# Problem dims (hardcoded; harness runs kernel.py standalone).
B, F, T, C = 2, 2048, 2048, 1024
H, D = 16, 64
HD = H * D
ALPHA = 1.0 / np.sqrt(np.float32(D)).astype(np.float32)
NEG = -100000.0
N_CORES = 8
NPAIR = (B * H) // N_CORES  # 4 (b,h) pairs per core
P = 128
DR = mybir.MatmulPerfMode.DoubleRow
WS = 64.0  # fp8 weight prescale (power of two; folded back at eviction)

REAL_DIMS = dict(npair=NPAIR, c=C, hd=HD, d=D, f=F, t=T)


def _nsl(total, step):
    """Split [0, total) into <=step slices (matmul moving free-dim limit)."""
    return [(s, min(s + step, total)) for s in range(0, total, step)]


def build_program_fast(dims=None, alpha=float(ALPHA)):
    dm = dims or REAL_DIMS
    npair, c, hd, d, f, t = (
        dm["npair"], dm["c"], dm["hd"], dm["d"], dm["f"], dm["t"],
    )
    bh = 2 * d
    ncc = c // P
    ncp = ncc // 2
    nch = t // P
    NB = 512
    fh = f // 2
    mult = mybir.AluOpType.mult
    subtract = mybir.AluOpType.subtract

    nc = bacc.Bacc(None, target_bir_lowering=False, debug=True)
    x8f = nc.declare_dram_parameter("x8f", [npair, P, ncc, bh], FP8, isOutput=False)
    xrf = nc.declare_dram_parameter("xrf", [npair, P, ncc, bh], FP8, isOutput=False)
    x8t = nc.declare_dram_parameter("x8t", [npair, P, ncc, bh], FP8, isOutput=False)
    xrt = nc.declare_dram_parameter("xrt", [npair, P, ncc, bh], FP8, isOutput=False)
    wparams = {}
    for name in ("q", "k", "v"):
        wparams[name] = (
            nc.declare_dram_parameter(f"w8{name}", [P, ncc, hd], FP8, isOutput=False),
            nc.declare_dram_parameter(f"wr{name}", [P, ncc, hd], FP8, isOutput=False),
        )
    out_d = nc.declare_dram_parameter("out", [npair, d + 1, f], F32, isOutput=True)

    with tile.TileContext(nc) as tc, ExitStack() as ctx:
        const = ctx.enter_context(tc.tile_pool(name="const", bufs=1))
        wpool = ctx.enter_context(tc.tile_pool(name="wpool", bufs=1))
        rqk = ctx.enter_context(tc.tile_pool(name="rqk", bufs=4))
        vpool = ctx.enter_context(tc.tile_pool(name="vpool", bufs=npair * nch))
        dpool = ctx.enter_context(tc.tile_pool(name="dpool", bufs=5, space="DRAM"))

        ident = const.tile([d + 1, d + 1], BF16)
        make_identity(nc, ident[:])

        w8_s, wr_s = {}, {}

        def load_weights():
            for name in ("q", "k", "v"):
                w8d, wrd = wparams[name]
                w8t_ = wpool.tile([P, ncc, hd], FP8, tag=f"w8{name}")
                wrt_ = wpool.tile([P, ncc, hd], FP8, tag=f"wr{name}")
                # per-chunk loads so the first projection matmul only waits
                # for one transfer, not the whole weight
                for kc in range(ncc):
                    nc.sync.dma_start(out=w8t_[:, kc, :], in_=w8d[:, kc, :])
                for kc in range(ncc):
                    nc.sync.dma_start(out=wrt_[:, kc, :], in_=wrd[:, kc, :])
                w8_s[name] = w8t_
                wr_s[name] = wrt_

        r_all = [{} for _ in range(npair)]
        vones_all = [[] for _ in range(npair)]

        xpool = ctx.enter_context(tc.tile_pool(name="xpool", bufs=2))
        blkpool = ctx.enter_context(tc.tile_pool(name="blkpool", bufs=4))
        rv = ctx.enter_context(tc.tile_pool(name="rv", bufs=2))
        epool = ctx.enter_context(tc.tile_pool(name="epool", bufs=12))
        opool = ctx.enter_context(tc.tile_pool(name="opool", bufs=2))
        # PSUM: 2 mix slots (128, hd) f32 [2 banks each] serve projections,
        # score halves and the transpose blocks; ctx accumulator [4 banks].
        pp_mix = ctx.enter_context(tc.tile_pool(name="pp_mix", bufs=2, space="PSUM"))
        pp_ctx = ctx.enter_context(tc.tile_pool(name="pp_ctx", bufs=1, space="PSUM"))

        def emit_proj(j):
            """fp8 projections + reshape bounce + v transposes for pair j."""
            x8f_s = xpool.tile([P, ncc, bh], FP8, tag="x8f")
            nc.sync.dma_start(out=x8f_s[:], in_=x8f[j])
            xrf_s = xpool.tile([P, ncc, bh], FP8, tag="xrf")
            nc.sync.dma_start(out=xrf_s[:], in_=xrf[j])
            x8t_s = xpool.tile([P, ncc, bh], FP8, tag="x8t")
            nc.sync.dma_start(out=x8t_s[:], in_=x8t[j])
            xrt_s = xpool.tile([P, ncc, bh], FP8, tag="xrt")
            nc.sync.dma_start(out=xrt_s[:], in_=xrt[j])
            yield
            for name, x8_s, xr_s in (
                ("q", x8f_s, xrf_s), ("k", x8t_s, xrt_s), ("v", x8t_s, xrt_s),
            ):
                pj = pp_mix.tile([bh, hd], F32, tag="mix")
                terms = (
                    (x8_s, w8_s[name]), (xr_s, w8_s[name]), (x8_s, wr_s[name]),
                )
                for ns, ne in _nsl(hd, NB):
                    for gi, (xs, ws) in enumerate(terms):
                        for cp in range(ncp):
                            nc.tensor.matmul(
                                pj[:, ns:ne],
                                xs[:, 2 * cp:2 * cp + 2, :],
                                ws[:, 2 * cp:2 * cp + 2, ns:ne],
                                start=(gi == 0 and cp == 0),
                                stop=(gi == 2 and cp == ncp - 1),
                                perf_mode=DR,
                            )
                    yield
                # evictions fold the 1/WS weight prescale back in; q/k are
                # split into fp8 value+residual pairs for the score matmul
                if name == "v":
                    blk = blkpool.tile([bh, hd], BF16, tag="blkv")
                    nc.vector.tensor_scalar_mul(blk[:], pj[:], 1.0 / WS)
                    dsc = dpool.tile([bh, hd], BF16, tag="dscv")
                    nc.sync.dma_start(out=dsc[:], in_=blk[:])
                    r = rv.tile([d + 1, 2 * hd], BF16, tag="rv")
                    # DRAM bounce realizes the raw (2d, hd)->(d, 2*hd)
                    # reshape: rows 2d', 2d'+1 are adjacent in DRAM
                    nc.sync.dma_start(
                        out=r[0:d, :],
                        in_=dsc[:].rearrange("(d two) n -> d (two n)", two=2),
                    )
                    r_all[j]["v"] = r
                else:
                    h8 = blkpool.tile([bh, hd], FP8, tag="h8")
                    nc.vector.tensor_scalar_mul(h8[:], pj[:], 1.0 / WS)
                    hr = blkpool.tile([bh, hd], FP8, tag="hr")
                    nc.vector.scalar_tensor_tensor(
                        hr[:], pj[:], 1.0 / WS, h8[:], op0=mult, op1=subtract,
                    )
                    d8 = dpool.tile([bh, hd], FP8, tag="d8")
                    nc.sync.dma_start(out=d8[:], in_=h8[:])
                    dr_ = dpool.tile([bh, hd], FP8, tag="dr")
                    nc.sync.dma_start(out=dr_[:], in_=hr[:])
                    d8r = d8[:].rearrange("(d two) n -> d (two n)", two=2)
                    drr = dr_[:].rearrange("(d two) n -> d (two n)", two=2)
                    if name == "q":
                        # moving side: [q8;qr] on partitions, duplicated
                        # across the two DoubleRow j-subtiles
                        s = rqk.tile([P, 2, f], FP8, tag="qs")
                        for jd in range(2):
                            nc.sync.dma_start(out=s[0:d, jd, :], in_=d8r)
                            nc.sync.dma_start(out=s[d:2 * d, jd, :], in_=drr)
                    else:
                        # stationary side: j=0 [k8;k8], j=1 [kr;kr]
                        s = rqk.tile([P, 2, t], FP8, tag="ks")
                        nc.sync.dma_start(out=s[0:d, 0, :], in_=d8r)
                        nc.sync.dma_start(out=s[d:2 * d, 0, :], in_=d8r)
                        nc.sync.dma_start(out=s[0:d, 1, :], in_=drr)
                        nc.sync.dma_start(out=s[d:2 * d, 1, :], in_=drr)
                    r_all[j][name] = s
                yield
            # ones row -> transposes carry the denominator column
            r_v = r_all[j]["v"]
            nc.vector.memset(r_v[d:d + 1, :], 1.0)
            grp = 4
            for tg in range(0, nch, grp):
                gn = min(grp, nch - tg)
                vt_ps = pp_mix.tile([P, grp, d + 2], BF16, tag="mix")
                for ti in range(gn):
                    tcb = tg + ti
                    nc.tensor.transpose(
                        vt_ps[:, ti, 0:d + 1],
                        r_v[:, tcb * P:(tcb + 1) * P],
                        ident[:],
                    )
                    vo = vpool.tile([P, d + 1], BF16, tag="vones")
                    nc.vector.tensor_copy(vo[:], vt_ps[:, ti, 0:d + 1])
                    vones_all[j].append(vo)
                yield

        def emit_attn(j):
            qs, ks = r_all[j]["q"], r_all[j]["k"]
            ps_cx = pp_ctx.tile([d + 1, f], F32, tag="cx")
            for tcb in range(nch):
                exs = []
                for hf in range(2):
                    ps_sc = pp_mix.tile([P, fh], F32, tag="mix")
                    for ns, ne in _nsl(fh, NB):
                        nc.tensor.matmul(
                            ps_sc[:, ns:ne],
                            ks[:, :, tcb * P:(tcb + 1) * P],
                            qs[:, :, hf * fh + ns:hf * fh + ne],
                            start=True, stop=True,
                            perf_mode=DR,
                        )
                    ex = epool.tile([P, fh], BF16, tag="exp")
                    nc.scalar.activation(
                        ex[:], ps_sc[:], mybir.ActivationFunctionType.Exp,
                        scale=alpha,
                    )
                    exs.append(ex)
                # PSUM accumulation groups work on 2KB zero regions (512
                # f32): start/stop on the first/last write of each region.
                REG = 512
                for hf in range(2):
                    for ns, ne in _nsl(fh, NB):
                        gs, ge = hf * fh + ns, hf * fh + ne
                        nc.tensor.matmul(
                            ps_cx[:, gs:ge],
                            vones_all[j][tcb][:],
                            exs[hf][:, ns:ne],
                            start=(tcb == 0 and gs % REG == 0),
                            stop=(tcb == nch - 1 and (ge % REG == 0 or ge == f)),
                        )
                yield
            # raw accumulator out; the denominator divide happens on host
            cx_sb = opool.tile([d + 1, f], F32, tag="ctx")
            nc.vector.tensor_copy(cx_sb[:], ps_cx[:])
            nc.sync.dma_start(out=out_d[j], in_=cx_sb[:])
            yield

        # software pipeline: pair j's attention interleaved (in program
        # order, hence in each engine's instruction stream) with pair j+1's
        # projections.
        pg0 = emit_proj(0)
        next(pg0)        # pair-0 x loads issue before the weight DMAs
        load_weights()
        for _ in pg0:
            pass
        for j in range(npair):
            pg = emit_proj(j + 1) if j + 1 < npair else None
            for _ in emit_attn(j):
                if pg is not None:
                    next(pg, None)
            if pg is not None:
                for _ in pg:
                    pass

    nc.finalize()
    return nc


def build_program_general(has_mask=False, has_bias=True, dims=None, exp_bufs=None):
    """All-bf16 fallback program (handles mask and bias)."""
    dm = dims or REAL_DIMS
    npair, c, hd, d, f, t = (
        dm["npair"], dm["c"], dm["hd"], dm["d"], dm["f"], dm["t"],
    )
    bh = 2 * d          # row-block height of x per (b,h) pair
    ncc = c // P        # contraction chunks for projections
    nch = t // P        # t' chunks for attention
    NB = 512            # matmul PSUM-write limit: one 2KB bank (512 f32)

    nc = bacc.Bacc(None, target_bir_lowering=False, debug=True)
    xfT = nc.declare_dram_parameter("xfT", [npair, P, ncc, bh], BF16, isOutput=False)
    xtT = nc.declare_dram_parameter("xtT", [npair, P, ncc, bh], BF16, isOutput=False)
    wq = nc.declare_dram_parameter("wq", [P, ncc, hd], BF16, isOutput=False)
    wk = nc.declare_dram_parameter("wk", [P, ncc, hd], BF16, isOutput=False)
    wv = nc.declare_dram_parameter("wv", [P, ncc, hd], BF16, isOutput=False)
    bq = nc.declare_dram_parameter("bq", [1, hd], BF16, isOutput=False)
    bk = nc.declare_dram_parameter("bk", [1, hd], BF16, isOutput=False)
    bv = nc.declare_dram_parameter("bv", [1, hd], BF16, isOutput=False)
    mbT = None
    if has_mask:
        mbT = nc.declare_dram_parameter("mbT", [t, f], BF16, isOutput=False)
    out_d = nc.declare_dram_parameter("out", [npair, d, f], F32, isOutput=True)

    with tile.TileContext(nc) as tc, ExitStack() as ctx:
        const = ctx.enter_context(tc.tile_pool(name="const", bufs=1))
        wpool = ctx.enter_context(tc.tile_pool(name="wpool", bufs=1))
        rqk = ctx.enter_context(tc.tile_pool(name="rqk", bufs=2 * npair))
        vpool = ctx.enter_context(tc.tile_pool(name="vpool", bufs=npair * nch))
        dpool = ctx.enter_context(tc.tile_pool(name="dpool", bufs=3, space="DRAM"))

        if has_bias:
            ones_row = const.tile([1, P], BF16)
            nc.vector.memset(ones_row[:], 1.0)
        ones_at_d = const.tile([d + 1, d], BF16)
        nc.vector.memset(ones_at_d[d:d + 1, :], 1.0)
        ident = const.tile([d + 1, d + 1], BF16)
        make_identity(nc, ident[:])

        w_s, b_s = {}, {}

        def load_weights():
            for name, wd, bd in (("q", wq, bq), ("k", wk, bk), ("v", wv, bv)):
                wt = wpool.tile([P, ncc, hd], BF16, tag=f"w{name}")
                for kc in range(ncc):
                    nc.sync.dma_start(out=wt[:, kc, :], in_=wd[:, kc, :])
                w_s[name] = wt
                if has_bias:
                    bt = wpool.tile([1, hd], BF16, tag=f"b{name}")
                    nc.sync.dma_start(out=bt[:], in_=bd[:])
                    b_s[name] = bt

        r_all = [{} for _ in range(npair)]
        vones_all = [[] for _ in range(npair)]
        cx_hold = {}
        fh = f // 2

        xpool = ctx.enter_context(tc.tile_pool(name="xpool", bufs=2))
        blkpool = ctx.enter_context(tc.tile_pool(name="blkpool", bufs=3))
        rv = ctx.enter_context(tc.tile_pool(name="rv", bufs=2))
        if exp_bufs is None:
            exp_bufs = 10 if has_mask else 12
        epool = ctx.enter_context(tc.tile_pool(name="epool", bufs=exp_bufs))
        opool = ctx.enter_context(tc.tile_pool(name="opool", bufs=2))
        spool = ctx.enter_context(tc.tile_pool(name="spool", bufs=1))
        mpool = None
        if has_mask:
            mpool = ctx.enter_context(tc.tile_pool(name="mpool", bufs=4))
        pp_mix = ctx.enter_context(tc.tile_pool(name="pp_mix", bufs=2, space="PSUM"))
        pp_ctx = ctx.enter_context(tc.tile_pool(name="pp_ctx", bufs=1, space="PSUM"))

        def emit_proj(j):
            xf_s = xpool.tile([P, ncc, bh], BF16, tag="xf")
            nc.sync.dma_start(out=xf_s[:], in_=xfT[j])
            xt_s = xpool.tile([P, ncc, bh], BF16, tag="xt")
            nc.sync.dma_start(out=xt_s[:], in_=xtT[j])
            yield
            for name, x_s in (("q", xf_s), ("k", xt_s), ("v", xt_s)):
                pj = pp_mix.tile([bh, hd], F32, tag="mix")
                if has_bias:
                    for ns, ne in _nsl(hd, NB):
                        nc.tensor.matmul(
                            pj[:, ns:ne], ones_row[:, :bh],
                            b_s[name][:, ns:ne],
                            start=True, stop=False,
                        )
                for kc in range(ncc):
                    first = kc == 0 and not has_bias
                    last = kc == ncc - 1
                    for ns, ne in _nsl(hd, NB):
                        nc.tensor.matmul(
                            pj[:, ns:ne], x_s[:, kc, :],
                            w_s[name][:, kc, ns:ne],
                            start=first, stop=last,
                        )
                    if kc % 3 == 2:
                        yield
                blk = blkpool.tile([bh, hd], BF16, tag="blk")
                if name == "k":
                    nc.vector.tensor_scalar_mul(blk[:], pj[:], float(ALPHA))
                else:
                    nc.vector.tensor_copy(blk[:], pj[:])
                dsc = dpool.tile([bh, hd], BF16, tag="dsc")
                nc.sync.dma_start(out=dsc[:], in_=blk[:])
                if name == "v":
                    r = rv.tile([d + 1, 2 * hd], BF16, tag="rv")
                else:
                    r = rqk.tile([d, 2 * hd], BF16, tag=f"r{name}")
                nc.sync.dma_start(
                    out=r[0:d, :],
                    in_=dsc[:].rearrange("(d two) n -> d (two n)", two=2),
                )
                r_all[j][name] = r
                yield
            r_v = r_all[j]["v"]
            nc.vector.memset(r_v[d:d + 1, :], 1.0)
            grp = 4
            for tg in range(0, nch, grp):
                gn = min(grp, nch - tg)
                vt_ps = pp_mix.tile([P, grp, d + 2], BF16, tag="mix")
                for ti in range(gn):
                    tcb = tg + ti
                    nc.tensor.transpose(
                        vt_ps[:, ti, 0:d + 1],
                        r_v[:, tcb * P:(tcb + 1) * P],
                        ident[:],
                    )
                    vo = vpool.tile([P, d + 1], BF16, tag="vones")
                    nc.vector.tensor_copy(vo[:], vt_ps[:, ti, 0:d + 1])
                    vones_all[j].append(vo)
                yield

        def emit_attn(j):
            r_q, r_k = r_all[j]["q"], r_all[j]["k"]
            ps_cx = pp_ctx.tile([d + 1, f], F32, tag="cx")
            for tcb in range(nch):
                exs = []
                for hf in range(2):
                    ps_sc = pp_mix.tile([P, fh], F32, tag="mix")
                    for ns, ne in _nsl(fh, NB):
                        nc.tensor.matmul(
                            ps_sc[:, ns:ne],
                            r_k[:, tcb * P:(tcb + 1) * P],
                            r_q[:, hf * fh + ns:hf * fh + ne],
                            start=True, stop=True,
                        )
                    if has_mask:
                        mt = mpool.tile([P, fh], BF16, tag="mb")
                        nc.sync.dma_start(
                            out=mt[:],
                            in_=mbT[tcb * P:(tcb + 1) * P, hf * fh:(hf + 1) * fh],
                        )
                        nc.vector.tensor_add(ps_sc[:], ps_sc[:], mt[:])
                    ex = epool.tile([P, fh], BF16, tag="exp")
                    nc.scalar.activation(
                        ex[:], ps_sc[:], mybir.ActivationFunctionType.Exp
                    )
                    exs.append(ex)
                REG = 512
                for hf in range(2):
                    for ns, ne in _nsl(fh, NB):
                        gs, ge = hf * fh + ns, hf * fh + ne
                        nc.tensor.matmul(
                            ps_cx[:, gs:ge],
                            vones_all[j][tcb][:],
                            exs[hf][:, ns:ne],
                            start=(tcb == 0 and gs % REG == 0),
                            stop=(tcb == nch - 1 and (ge % REG == 0 or ge == f)),
                        )
                yield
            cx_sb = opool.tile([d + 1, f], F32, tag="ctx")
            nc.vector.tensor_copy(cx_sb[:], ps_cx[:])
            cx_hold[j] = cx_sb
            yield

        def emit_norm(j):
            cx_sb = cx_hold[j]
            nc.vector.reciprocal(cx_sb[d:d + 1, :], cx_sb[d:d + 1, :])
            rc_bf = spool.tile([d + 1, f], BF16, tag="rcb")
            nc.vector.tensor_copy(rc_bf[d:d + 1, :], cx_sb[d:d + 1, :])
            yield
            bc_sb = spool.tile([d, f], F32, tag="bc")
            for hs, he in _nsl(f, min(fh, 1024)):
                ps_bc = pp_mix.tile([d, min(fh, 1024)], F32, tag="mix")
                for ns, ne in _nsl(he - hs, NB):
                    nc.tensor.matmul(
                        ps_bc[:, ns:ne], ones_at_d[d:d + 1, :],
                        rc_bf[d:d + 1, hs + ns:hs + ne],
                        start=True, stop=True,
                    )
                nc.vector.tensor_copy(bc_sb[:, hs:he], ps_bc[:, 0:he - hs])
                yield
            nc.vector.tensor_mul(cx_sb[0:d, :], cx_sb[0:d, :], bc_sb[:])
            nc.sync.dma_start(out=out_d[j], in_=cx_sb[0:d, :])
            yield

        pg0 = emit_proj(0)
        next(pg0)
        load_weights()
        for _ in pg0:
            pass
        ng = None
        for j in range(npair):
            pg = emit_proj(j + 1) if j + 1 < npair else None
            for _ in emit_attn(j):
                if pg is not None:
                    next(pg, None)
                if ng is not None:
                    next(ng, None)
            if pg is not None:
                for _ in pg:
                    pass
            if ng is not None:
                for _ in ng:
                    pass
            ng = emit_norm(j)
        for _ in ng:
            pass

    nc.finalize()
    return nc


_PROGRAM_CACHE = {}
TRACE = False
LAST_RESULTS = None


def _get_program(key):
    if key not in _PROGRAM_CACHE:
        if key == "fast":
            _PROGRAM_CACHE[key] = build_program_fast()
        else:
            has_mask, has_bias = key
            _PROGRAM_CACHE[key] = build_program_general(
                has_mask=has_mask, has_bias=has_bias
            )
    return _PROGRAM_CACHE[key]


def _split8(a):
    a8 = a.astype(NP_FP8)
    ar = (a - a8.astype(np.float32)).astype(NP_FP8)
    return a8, ar


def _kernel_fast(inputs, from_tensor, to_tensor):
    nc = _get_program("fast")
    bh = 2 * D

    def lay(a, inner):
        return np.ascontiguousarray(
            a.reshape(C // 128, 128, inner).transpose(1, 0, 2)
        )

    wmaps = {}
    for name, key in (("q", "Wq"), ("k", "Wk"), ("v", "Wv")):
        w8, wr = _split8(np.asarray(inputs[key], np.float32) * WS)
        wmaps[f"w8{name}"] = lay(w8, HD)
        wmaps[f"wr{name}"] = lay(wr, HD)

    def xprep(x, p):
        xb = np.ascontiguousarray(
            x[p // H, (p % H) * bh:(p % H + 1) * bh, :].T
        ).astype(np.float32)
        x8, xr = _split8(xb)
        return lay(x8, bh), lay(xr, bh)

    in_maps = []
    for core in range(N_CORES):
        pairs = [NPAIR * core + jj for jj in range(NPAIR)]
        x8f = np.empty((NPAIR, 128, C // 128, bh), NP_FP8)
        xrf = np.empty_like(x8f)
        x8t = np.empty_like(x8f)
        xrt = np.empty_like(x8f)
        for jj, p in enumerate(pairs):
            x8f[jj], xrf[jj] = xprep(from_tensor, p)
            x8t[jj], xrt[jj] = xprep(to_tensor, p)
        m = {"x8f": x8f, "xrf": xrf, "x8t": x8t, "xrt": xrt}
        m.update(wmaps)
        in_maps.append(m)

    res = run_bass_kernel_spmd(
        nc, in_maps, core_ids=list(range(N_CORES)), trace=TRACE
    )
    global LAST_RESULTS
    LAST_RESULTS = res

    out = np.empty((B, HD, F), np.float32)
    for core in range(N_CORES):
        o = res.results[core]["out"]  # (NPAIR, D+1, F)
        for jj in range(NPAIR):
            p = NPAIR * core + jj
            b, h = p // H, p % H
            out[b, h * D:(h + 1) * D, :] = o[jj][:D] / o[jj][D:D + 1]
    return out


def _kernel_general(inputs, from_tensor, to_tensor, mb, has_mask, has_bias):
    nc = _get_program((has_mask, has_bias))
    bh = 2 * D

    def wprep(w):
        w = np.asarray(w, np.float32).astype(NP_BF16)
        return np.ascontiguousarray(
            w.reshape(C // 128, 128, HD).transpose(1, 0, 2)
        )

    wq = wprep(inputs["Wq"])
    wk = wprep(inputs["Wk"])
    wv = wprep(inputs["Wv"])
    bqv = np.asarray(inputs["bq"], np.float32).astype(NP_BF16).reshape(1, HD)
    bkv = np.asarray(inputs["bk"], np.float32).astype(NP_BF16).reshape(1, HD)
    bvv = np.asarray(inputs["bv"], np.float32).astype(NP_BF16).reshape(1, HD)

    def xprep(x, p):
        xb = x[p // H, (p % H) * bh:(p % H + 1) * bh, :].T.astype(NP_BF16)
        return np.ascontiguousarray(
            xb.reshape(C // 128, 128, bh).transpose(1, 0, 2)
        )

    in_maps = []
    for core in range(N_CORES):
        pairs = [NPAIR * core + jj for jj in range(NPAIR)]
        b = pairs[0] // H
        xf = np.stack([xprep(from_tensor, p) for p in pairs])
        xt = np.stack([xprep(to_tensor, p) for p in pairs])
        m = {
            "xfT": xf, "xtT": xt,
            "wq": wq, "wk": wk, "wv": wv,
            "bq": bqv, "bk": bkv, "bv": bvv,
        }
        if has_mask:
            m["mbT"] = np.ascontiguousarray(mb[b].T).astype(NP_BF16)
        in_maps.append(m)

    res = run_bass_kernel_spmd(
        nc, in_maps, core_ids=list(range(N_CORES)), trace=TRACE
    )
    global LAST_RESULTS
    LAST_RESULTS = res

    out = np.empty((B, HD, F), np.float32)
    for core in range(N_CORES):
        o = res.results[core]["out"]
        for jj in range(NPAIR):
            p = NPAIR * core + jj
            b, h = p // H, p % H
            out[b, h * D:(h + 1) * D, :] = o[jj]
    return out


def kernel(**inputs):
    from_tensor = np.asarray(inputs["from_tensor"], np.float32)
    to_tensor = np.asarray(inputs["to_tensor"], np.float32)
    mask = np.asarray(inputs["mask"], np.float32)

    mb = (1.0 - mask) * NEG  # (B, F, T) additive mask bias
    has_mask = bool(np.any(mb != 0.0))
    has_bias = bool(
        np.any(inputs["bq"]) or np.any(inputs["bk"]) or np.any(inputs["bv"])
    )
    if not has_mask and not has_bias:
        return _kernel_fast(inputs, from_tensor, to_tensor)
    return _kernel_general(
        inputs, from_tensor, to_tensor, mb, has_mask, has_bias
    )
